# revision 10
# baseline (speedup 1.0000x reference)
"""Distributed GCN (2x GCNConv + global_mean_pool + linear head) on 8 Trainium2
NeuronCores via Bass/Tile.

Sharding: nodes are split into 8 contiguous ranges; each core owns the edges
whose *destination* falls in its range.  Weights are replicated.  Per layer
each core computes g = dinv * (h @ W) for its own node slice, the slices are
AllGathered into a full gather table in HBM, the core then gathers g[src] for
its edges with dma_gather (two <=32768-row table halves, int16 indices) and
reduces them per 256-node dst window with one-hot-matrix matmuls accumulated in
PSUM (segmented scatter-add as matmul).  Self-loop terms enter the same PSUM
accumulation as PE transposes of the core's own g rows.  Pooled sums/counts
are AllReduced at the end.
"""

import math
import os
import sys

import numpy as np

for _p in ("/opt/trn_rl_repo", "/root/.axon_site/_ro/trn_rl_repo"):
    if os.path.isdir(_p) and _p not in sys.path:
        sys.path.append(_p)

import concourse.bacc as bacc
import concourse.bass as bass
import concourse.tile as tile
from concourse import mybir
from concourse.masks import make_identity

F = 64            # feature/hidden width
P = 128           # partitions
WIN = 256         # dst-window (PSUM segment) size in nodes
CHUNK_TOK = 4096  # gather tokens per dma_gather call
SBATCH = 4        # selection-matrix tiles built per DVE op


class Cfg:
    def __init__(self, n_nodes=50000, n_edges=800000, n_graphs=512, n_cores=8):
        assert n_nodes % n_cores == 0
        self.n_nodes = n_nodes
        self.n_edges = n_edges
        self.n_graphs = n_graphs
        self.n_cores = n_cores
        self.npc = n_nodes // n_cores             # nodes per core
        self.nwa = math.ceil(self.npc / WIN)      # agg windows per core
        self.nwc = math.ceil(self.npc / P)        # 128-col windows per core


# ---------------------------------------------------------------------------
# host-side graph partitioning (integer/structural work only)
# ---------------------------------------------------------------------------

def host_prep(cfg: Cfg, edge_index: np.ndarray, batch: np.ndarray):
    N, C, NPC, NWA = cfg.n_nodes, cfg.n_cores, cfg.npc, cfg.nwa
    NWC = cfg.nwc
    HALF = N // 2
    assert HALF <= 32768 and N - HALF <= 32768
    src = edge_index[0].astype(np.int64)
    dst = edge_index[1].astype(np.int64)

    # reference degree includes the self-loop; the self-loop term itself is
    # applied on-device as a transpose of the core's own g rows.
    deg = (np.bincount(dst, minlength=N) + 1).astype(np.float32)

    core_of = dst // NPC
    wloc = (dst - core_of * NPC) // WIN
    half_of = (src >= HALF).astype(np.int64)

    keys = (core_of * NWA + wloc) * 2 + half_of
    order = np.argsort(keys, kind="stable")
    s_sorted = src[order]
    d_sorted = dst[order]
    counts = np.bincount(keys[order], minlength=C * NWA * 2).reshape(C, NWA, 2)
    starts = np.zeros(C * NWA * 2 + 1, dtype=np.int64)
    np.cumsum(counts.reshape(-1), out=starts[1:])

    # tiles per (window, half), equalized across cores (single SPMD program)
    nt2 = np.ceil(counts / P).astype(np.int64).max(axis=0)        # [NWA, 2]
    meta = dict(nt=[], tile_base=[], T=[], chunks=[])
    per_core = dict(idx16=[], dst_rel=[])
    ch_cols = CHUNK_TOK // P
    for h in range(2):
        nt = nt2[:, h]
        tile_base = np.zeros(NWA + 1, dtype=np.int64)
        np.cumsum(nt, out=tile_base[1:])
        T = int(tile_base[-1])
        L = T * P
        idx16 = np.zeros((C, 128, max(1, T * 8)), dtype=np.int16)
        dst_rel = np.full((C, P, max(1, T)), -1.0, dtype=np.float32)
        for c in range(C):
            flat_idx = np.zeros(max(16, L), dtype=np.int16)
            for w in range(NWA):
                cnt = int(counts[c, w, h])
                if cnt == 0:
                    continue
                e0 = int(starts[(c * NWA + w) * 2 + h])
                tok = tile_base[w] * P + np.arange(cnt)
                flat_idx[tok] = (s_sorted[e0:e0 + cnt] - h * HALF).astype(
                    np.int16)
                dst_rel[c, tok % P, tok // P] = (
                    d_sorted[e0:e0 + cnt] - c * NPC - w * WIN
                ).astype(np.float32)
            if L > 0:
                wrapped = flat_idx[:L].reshape(L // 16, 16).T      # [16, L/16]
                idx16[c] = np.tile(wrapped, (8, 1))
        meta["nt"].append(nt.tolist())
        meta["tile_base"].append(tile_base.tolist())
        meta["T"].append(T)
        meta["chunks"].append(
            [(c0, min(c0 + ch_cols, T)) for c0 in range(0, T, ch_cols)])
        per_core["idx16"].append(idx16)
        per_core["dst_rel"].append(dst_rel)

    # batch ids per own node, node-major [P, NWC], -1 for tail padding
    batch_col = np.full((C, P, NWC), -1.0, dtype=np.float32)
    for c in range(C):
        own = batch[c * NPC:(c + 1) * NPC].astype(np.float32)
        n = np.arange(NPC)
        batch_col[c, n % P, n // P] = own

    # deg for own nodes in both layouts (pad with 1.0 -> dinv finite)
    deg_col = np.ones((C, P, NWC), dtype=np.float32)
    deg_row = np.ones((C, 1, NWA * WIN), dtype=np.float32)
    for c in range(C):
        own = deg[c * NPC:(c + 1) * NPC]
        n = np.arange(NPC)
        deg_col[c, n % P, n // P] = own
        deg_row[c, 0, :NPC] = own

    meta["half"] = HALF
    per_core.update(batch_col=batch_col, deg_col=deg_col, deg_row=deg_row)
    return meta, per_core


# ---------------------------------------------------------------------------
# Bass program
# ---------------------------------------------------------------------------

def build_program(cfg: Cfg, meta, debug_outputs: bool = False):
    N, C, NPC, G = cfg.n_nodes, cfg.n_cores, cfg.npc, cfg.n_graphs
    NWA, NWC = cfg.nwa, cfg.nwc
    HALF = meta["half"]
    T2 = meta["T"]
    nt2 = meta["nt"]
    tile_base2 = meta["tile_base"]
    chunks2 = meta["chunks"]
    f32 = mybir.dt.float32
    i16 = mybir.dt.int16
    FT = mybir.ActivationFunctionType
    ALU = mybir.AluOpType
    GI = max(G, WIN)                      # iota width for S builds + pooling
    ch_cols = CHUNK_TOK // P

    nc = bacc.Bacc("TRN2", target_bir_lowering=False, debug=False,
                   num_devices=C)

    # ---- I/O ----
    xT_d = nc.dram_tensor("xT", [F, NPC], f32, kind="ExternalInput")
    W1_d = nc.dram_tensor("W1", [F, F], f32, kind="ExternalInput")
    W2_d = nc.dram_tensor("W2", [F, F], f32, kind="ExternalInput")
    b1_d = nc.dram_tensor("b1", [F, 1], f32, kind="ExternalInput")
    b2_d = nc.dram_tensor("b2", [F, 1], f32, kind="ExternalInput")
    woutf_d = nc.dram_tensor("wout_f", [F, 1], f32, kind="ExternalInput")
    wlast_d = nc.dram_tensor("wlast", [1, 1], f32, kind="ExternalInput")
    bout_d = nc.dram_tensor("bout", [1, 1], f32, kind="ExternalInput")
    depth_d = nc.dram_tensor("depth_row", [1, G], f32, kind="ExternalInput")
    degc_d = nc.dram_tensor("deg_col", [P, NWC], f32, kind="ExternalInput")
    degr_d = nc.dram_tensor("deg_row", [1, NWA * WIN], f32,
                            kind="ExternalInput")
    iota_d = nc.dram_tensor("iota_all", [P, GI], f32, kind="ExternalInput")
    idx_d = [nc.dram_tensor(f"idx16_{h}", [128, max(1, T2[h] * 8)], i16,
                            kind="ExternalInput") for h in range(2)]
    drel_d = [nc.dram_tensor(f"dst_rel_{h}", [P, max(1, T2[h])], f32,
                             kind="ExternalInput") for h in range(2)]
    bcol_d = nc.dram_tensor("batch_col", [P, NWC], f32, kind="ExternalInput")
    y_d = nc.dram_tensor("y_out", [1, G], f32, kind="ExternalOutput")
    dbg = {}
    if debug_outputs:
        dbg["h1T"] = nc.dram_tensor("dbg_h1T", [F, NPC], f32,
                                    kind="ExternalOutput")
        dbg["h2T"] = nc.dram_tensor("dbg_h2T", [F, NPC], f32,
                                    kind="ExternalOutput")
        dbg["g1"] = nc.dram_tensor("dbg_g1", [N, F], f32,
                                   kind="ExternalOutput")
        dbg["pool"] = nc.dram_tensor("dbg_pool", [2, G], f32,
                                     kind="ExternalOutput")

    with tile.TileContext(nc) as tc:
        with (
            tc.tile_pool(name="const", bufs=1) as const_pool,
            tc.tile_pool(name="big", bufs=1) as big_pool,
            tc.tile_pool(name="gbuf", bufs=4) as gbuf_pool,
            tc.tile_pool(name="work", bufs=2) as work_pool,
            tc.tile_pool(name="spool", bufs=2) as s_pool,
            tc.tile_pool(name="psA", bufs=2, space="PSUM") as psumA,
            tc.tile_pool(name="psB", bufs=2, space="PSUM") as psumB,
            tc.tile_pool(name="psC", bufs=2, space="PSUM") as psumC,
            tc.tile_pool(name="dram", bufs=1, space="DRAM") as dram_pool,
        ):
            # ---- load constants ----
            def load(pool, dram_t, shape, dtype=f32, name=None):
                t = pool.tile(shape, dtype, name=name or dram_t.name + "_sb")
                nc.sync.dma_start(t[:], dram_t[:])
                return t

            xT = load(big_pool, xT_d, [F, NPC])
            W1 = load(const_pool, W1_d, [F, F])
            W2 = load(const_pool, W2_d, [F, F])
            b1 = load(const_pool, b1_d, [F, 1])
            b2 = load(const_pool, b2_d, [F, 1])
            woutf = load(const_pool, woutf_d, [F, 1])
            wlast = load(const_pool, wlast_d, [1, 1])
            bout = load(const_pool, bout_d, [1, 1])
            depth = load(const_pool, depth_d, [1, G])
            deg_col = load(const_pool, degc_d, [P, NWC])
            deg_row = load(const_pool, degr_d, [1, NWA * WIN])
            iota = load(const_pool, iota_d, [P, GI])
            idx_sb = [load(big_pool, idx_d[h], [128, max(1, T2[h] * 8)],
                           i16, name=f"idx_sb{h}") for h in range(2)]
            drel = [load(big_pool, drel_d[h], [P, max(1, T2[h])],
                         name=f"drel_sb{h}") for h in range(2)]
            bcol = load(const_pool, bcol_d, [P, NWC])

            ones1F = const_pool.tile([1, F], f32, name="ones1F")
            nc.vector.memset(ones1F[:], 1.0)
            ident = const_pool.tile([P, P], f32, name="ident")
            make_identity(nc, ident[:])

            # ---- dinv: node-major for the linear phase ----
            dinv_col = const_pool.tile([P, NWC], f32, name="dinv_col")
            nc.scalar.activation(dinv_col[:], deg_col[:], FT.Sqrt)
            nc.vector.reciprocal(dinv_col[:], dinv_col[:])

            # ---- dinv broadcast across features: bcast deg then sqrt+recip
            dinvT = big_pool.tile([F, NWA * WIN], f32, name="dinvT")
            for j0 in range(0, NWA * WIN, 512):
                j1 = min(j0 + 512, NWA * WIN)
                ps = psumB.tile([F, 512], f32, name="bc_ps", tag="psB")
                nc.tensor.matmul(ps[:, : j1 - j0], ones1F[:],
                                 deg_row[:, j0:j1], start=True, stop=True)
                nc.scalar.activation(dinvT[:, j0:j1], ps[:, : j1 - j0],
                                     FT.Sqrt)
                nc.vector.reciprocal(dinvT[:, j0:j1], dinvT[:, j0:j1])

            # ---- internal DRAM gather tables ----
            g1_own = dram_pool.tile([NPC, F], f32, name="g1_own")
            g2_own = dram_pool.tile([NPC, F], f32, name="g2_own")
            g1_full = dram_pool.tile([N, F], f32, name="g1_full",
                                     addr_space="Shared")
            g2_full = dram_pool.tile([N, F], f32, name="g2_full",
                                     addr_space="Shared")
            cc_in = dram_pool.tile([2, G], f32, name="cc_in")
            cc_out = dram_pool.tile([2, G], f32, name="cc_out",
                                    addr_space="Shared")

            # ---- linear phase: g_own = dinv * (h @ W), h given transposed
            def linear_phase(hT, W, g_own_dram):
                for w in range(NWC):
                    n0 = w * P
                    n1 = min(n0 + P, NPC)
                    m = n1 - n0
                    ps = psumA.tile([P, F], f32, name="lin_ps", tag="psA")
                    nc.tensor.matmul(ps[:m, :], hT[:, n0:n1], W[:],
                                     start=True, stop=True)
                    gt = work_pool.tile([P, F], f32, name="lin_g")
                    nc.vector.tensor_scalar(gt[:m, :], ps[:m, :],
                                            dinv_col[:m, w:w + 1], None,
                                            op0=ALU.mult)
                    nc.sync.dma_start(g_own_dram[n0:n1, :], gt[:m, :])

            def allgather(g_own, g_full):
                nc.gpsimd.collective_compute(
                    "AllGather", ALU.bypass,
                    replica_groups=[list(range(C))],
                    ins=[g_own.opt()], outs=[g_full.opt()],
                )

            # ---- pooling window op (layer 2 only), per 128-node subwindow
            pool_ps = psumC.tile([2, G], f32, name="pool_ps", tag="psC")

            def pool_sub(h2T, s):
                n0 = s * P
                m = min(P, NPC - n0)
                sc_ps = psumC.tile([P, 1], f32, name="score_ps", tag="psD")
                nc.tensor.matmul(sc_ps[:m, :], h2T[:, n0:n0 + m], woutf[:],
                                 start=True, stop=True)
                sc = work_pool.tile([P, 2], f32, name="score_sb")
                nc.vector.memset(sc[:], 0.0)
                nc.vector.memset(sc[:m, 1:2], 1.0)
                nc.vector.tensor_copy(sc[:m, 0:1], sc_ps[:m, :])
                sg = work_pool.tile([P, G], f32, name="sel_pool")
                nc.vector.tensor_scalar(sg[:], iota[:, :G], bcol[:, s:s + 1],
                                        None, op0=ALU.is_equal)
                nc.tensor.matmul(pool_ps[:], sc[:], sg[:],
                                 start=(s == 0), stop=(s == NWC - 1))

            # ---- aggregation: hT = relu(dinv * (scatter_add(g[src]) + g_own)
            #      + b), where the g_own term is the self-loop contribution.
            def agg_phase(g_full, g_own, b_tile, hT_out, suffix, do_pool):
                tabs = [g_full[0:HALF, :], g_full[HALF:N, :]]
                chunk_tiles = {}

                def ensure_chunk(h, ci):
                    key = (h, ci)
                    if key in chunk_tiles:
                        return chunk_tiles[key]
                    c0, c1 = chunks2[h][ci]
                    ntok = (c1 - c0) * P
                    ct = gbuf_pool.tile([P, ch_cols * F], f32,
                                        name="chunk_" + suffix, tag="chunk")
                    nc.gpsimd.dma_gather(
                        out_ap=ct[:, :(c1 - c0) * F].rearrange(
                            "p (s e) -> p s e", e=F),
                        in_ap=tabs[h],
                        idxs_ap=idx_sb[h][:, c0 * 8:c1 * 8],
                        num_idxs=ntok,
                        num_idxs_reg=ntok,
                        elem_size=F,
                        single_packet=False,
                    )
                    chunk_tiles[key] = (ct, c0)
                    return chunk_tiles[key]

                for w in range(NWA):
                    n0 = w * WIN
                    m = min(WIN, NPC - n0)
                    n_tr = (m + P - 1) // P
                    ps = psumA.tile([F, WIN], f32, name="agg_ps_" + suffix,
                                    tag="psA")
                    nmm = nt2[0][w] + nt2[1][w] + n_tr
                    i = 0
                    for h in range(2):
                        ntw = nt2[h][w]
                        base = tile_base2[h][w]
                        for b0 in range(0, ntw, SBATCH):
                            nb = min(SBATCH, ntw - b0)
                            sw = s_pool.tile([P, nb * WIN], f32,
                                             name="sel_" + suffix, tag="sel",
                                             padded_shape=[P, SBATCH * WIN])
                            nc.vector.tensor_tensor(
                                sw[:].rearrange("p (t j) -> p t j", j=WIN),
                                iota[:, :WIN].rearrange(
                                    "p (o j) -> p o j", o=1).to_broadcast(
                                        (P, nb, WIN)),
                                drel[h][:, base + b0:base + b0 + nb].rearrange(
                                    "p (t o) -> p t o", o=1).to_broadcast(
                                        (P, nb, WIN)),
                                op=ALU.is_equal)
                            for t in range(nb):
                                gt = base + b0 + t
                                ct, c0 = ensure_chunk(h, gt // ch_cols)
                                col = gt - c0
                                nc.tensor.matmul(
                                    ps[:], ct[:, col * F:(col + 1) * F],
                                    sw[:, t * WIN:(t + 1) * WIN],
                                    start=(i == 0), stop=False)
                                i += 1
                    # self-loop term: transpose own g rows into the window
                    for s in range(n_tr):
                        ms = min(P, m - s * P)
                        gsl = work_pool.tile([P, F], f32,
                                             name="gself_" + suffix)
                        nc.sync.dma_start(
                            gsl[:ms, :], g_own[n0 + s * P:n0 + s * P + ms, :])
                        i += 1
                        nc.tensor.matmul(
                            ps[:, s * P:s * P + ms], gsl[:ms, :],
                            ident[:ms, :ms], is_transpose=True,
                            start=False, stop=(i == nmm))
                    tmp = work_pool.tile([F, WIN], f32, name="fin_" + suffix)
                    nc.vector.tensor_tensor(
                        tmp[:, :m], ps[:, :m], dinvT[:, n0:n0 + m],
                        op=ALU.mult)
                    nc.scalar.activation(hT_out[:, n0:n0 + m], tmp[:, :m],
                                         FT.Relu, bias=b_tile[:])
                    if do_pool:
                        for s in range(n0 // P, (n0 + m + P - 1) // P):
                            pool_sub(hT_out, s)

            h1T = big_pool.tile([F, NPC], f32, name="h1T")
            h2T = big_pool.tile([F, NPC], f32, name="h2T", tag="xT_sb")

            # ---- layer 1 ----
            linear_phase(xT, W1, g1_own)
            allgather(g1_own, g1_full)
            agg_phase(g1_full, g1_own, b1, h1T, "l1", do_pool=False)

            # ---- layer 2 (pooling fused into the window loop) ----
            linear_phase(h1T, W2, g2_own)
            allgather(g2_own, g2_full)
            agg_phase(g2_full, g2_own, b2, h2T, "l2", do_pool=True)

            # ---- pooled sums/counts AllReduce + head ----
            pool_sb = const_pool.tile([2, G], f32, name="pool_sb")
            nc.vector.tensor_copy(pool_sb[:], pool_ps[:])
            nc.sync.dma_start(cc_in[:], pool_sb[:])
            nc.gpsimd.collective_compute(
                "AllReduce", ALU.add, replica_groups=[list(range(C))],
                ins=[cc_in.opt()], outs=[cc_out.opt()])
            pool_g0 = const_pool.tile([1, G], f32, name="pool_g0")
            pool_g1 = const_pool.tile([1, G], f32, name="pool_g1")
            nc.sync.dma_start(pool_g0[:], cc_out[0:1, :])
            nc.sync.dma_start(pool_g1[:], cc_out[1:2, :])
            if debug_outputs:
                nc.sync.dma_start(dbg["pool"][:], cc_out[:])

            # y = sums/max(cnt,1) + depth*wlast + bout
            cnt = const_pool.tile([1, G], f32, name="cnt_row")
            nc.vector.tensor_scalar(cnt[:], pool_g1[:], 1.0, None,
                                    op0=ALU.max)
            nc.vector.reciprocal(cnt[:], cnt[:])
            y = const_pool.tile([1, G], f32, name="y_row")
            nc.vector.tensor_tensor(y[:], pool_g0[:], cnt[:], op=ALU.mult)
            dterm = const_pool.tile([1, G], f32, name="dterm")
            nc.vector.tensor_scalar(dterm[:], depth[:], wlast[:], None,
                                    op0=ALU.mult)
            nc.vector.tensor_tensor(y[:], y[:], dterm[:], op=ALU.add)
            nc.vector.tensor_scalar(y[:], y[:], bout[:], None, op0=ALU.add)
            nc.sync.dma_start(y_d[:], y[:])

            if debug_outputs:
                nc.sync.dma_start(dbg["h1T"][:], h1T[:])
                nc.sync.dma_start(dbg["h2T"][:], h2T[:])
                nc.gpsimd.dma_start(dbg["g1"][:], g1_full[:])

    nc.compile()
    return nc


# ---------------------------------------------------------------------------
# full pipeline
# ---------------------------------------------------------------------------

def make_in_maps(cfg: Cfg, meta, per_core, x, depth, W1, b1, W2, b2, Wout,
                 bout):
    C, NPC, G = cfg.n_cores, cfg.npc, cfg.n_graphs
    GI = max(G, WIN)
    iota = np.broadcast_to(np.arange(GI, dtype=np.float32), (P, GI)).copy()
    in_maps = []
    for c in range(C):
        xT = np.ascontiguousarray(x[c * NPC:(c + 1) * NPC, :].T)
        in_maps.append({
            "xT": xT,
            "W1": np.ascontiguousarray(W1),
            "W2": np.ascontiguousarray(W2),
            "b1": b1.reshape(F, 1).copy(),
            "b2": b2.reshape(F, 1).copy(),
            "wout_f": Wout[:F, :].copy(),
            "wlast": Wout[F:, :].copy(),
            "bout": bout.reshape(1, 1).copy(),
            "depth_row": depth.reshape(1, G).copy(),
            "deg_col": per_core["deg_col"][c],
            "deg_row": per_core["deg_row"][c],
            "iota_all": iota,
            "idx16_0": per_core["idx16"][0][c],
            "idx16_1": per_core["idx16"][1][c],
            "dst_rel_0": per_core["dst_rel"][0][c],
            "dst_rel_1": per_core["dst_rel"][1][c],
            "batch_col": per_core["batch_col"][c],
        })
    return in_maps


def kernel(x, edge_index, batch, depth, W1, b1, W2, b2, Wout, bout):
    cfg = Cfg()
    x = np.asarray(x, dtype=np.float32)
    edge_index = np.asarray(edge_index)
    batch = np.asarray(batch)
    depth = np.asarray(depth, dtype=np.float32)
    W1 = np.asarray(W1, dtype=np.float32)
    b1 = np.asarray(b1, dtype=np.float32)
    W2 = np.asarray(W2, dtype=np.float32)
    b2 = np.asarray(b2, dtype=np.float32)
    Wout = np.asarray(Wout, dtype=np.float32)
    bout = np.asarray(bout, dtype=np.float32)

    meta, per_core = host_prep(cfg, edge_index, batch)
    nc = build_program(cfg, meta)
    in_maps = make_in_maps(cfg, meta, per_core, x, depth, W1, b1, W2, b2,
                           Wout, bout)
    from concourse import bass_utils
    res = bass_utils.run_bass_kernel_spmd(
        nc, in_maps, core_ids=list(range(cfg.n_cores)))
    y = np.asarray(res.results[0]["y_out"]).reshape(cfg.n_graphs)
    return y.astype(np.float32)


if __name__ == "__main__":
    sys.path.insert(0, os.path.dirname(os.path.abspath(__file__)))
    import reference
    inputs = {k: np.asarray(v) for k, v in reference.setup_inputs().items()}
    out = kernel(**inputs)
    print("kernel output:", out[:8])


# revision 11
# speedup vs baseline: 1.1949x; 1.1949x over previous
"""Distributed GCN (2x GCNConv + global_mean_pool + linear head) on 8 Trainium2
NeuronCores via Bass/Tile.

Sharding: nodes are split into 8 contiguous ranges; each core owns the edges
whose *destination* falls in its range.  Weights are replicated.  Per layer
each core computes g = dinv * (h @ W) for its own node slice, the slices are
AllGathered into a full gather table in HBM, the core then gathers g[src] for
its edges with dma_gather (two <=32768-row table halves, int16 indices) and
reduces them per 256-node dst window with one-hot-matrix matmuls accumulated in
PSUM (segmented scatter-add as matmul).  Self-loop terms enter the same PSUM
accumulation as PE transposes of the core's own g rows.  Pooled sums/counts
are AllReduced at the end.
"""

import math
import os
import sys

import numpy as np

for _p in ("/opt/trn_rl_repo", "/root/.axon_site/_ro/trn_rl_repo"):
    if os.path.isdir(_p) and _p not in sys.path:
        sys.path.append(_p)

import concourse.bacc as bacc
import concourse.bass as bass
import concourse.tile as tile
from concourse import mybir
from concourse.masks import make_identity

F = 64            # feature/hidden width
P = 128           # partitions
WIN = 256         # dst-window (PSUM segment) size in nodes
CHUNK_TOK = 4096  # gather tokens per dma_gather call
SBATCH = 4        # selection-matrix tiles built per DVE op


class Cfg:
    def __init__(self, n_nodes=50000, n_edges=800000, n_graphs=512, n_cores=8):
        assert n_nodes % n_cores == 0
        self.n_nodes = n_nodes
        self.n_edges = n_edges
        self.n_graphs = n_graphs
        self.n_cores = n_cores
        self.npc = n_nodes // n_cores             # nodes per core
        self.nwa = math.ceil(self.npc / WIN)      # agg windows per core
        self.nwc = math.ceil(self.npc / P)        # 128-col windows per core


# ---------------------------------------------------------------------------
# host-side graph partitioning (integer/structural work only)
# ---------------------------------------------------------------------------

def host_prep(cfg: Cfg, edge_index: np.ndarray, batch: np.ndarray):
    N, C, NPC, NWA = cfg.n_nodes, cfg.n_cores, cfg.npc, cfg.nwa
    NWC = cfg.nwc
    HALF = N // 2
    assert HALF <= 32768 and N - HALF <= 32768
    src = edge_index[0].astype(np.int64)
    dst = edge_index[1].astype(np.int64)

    # reference degree includes the self-loop; the self-loop term itself is
    # applied on-device as a transpose of the core's own g rows.
    deg = (np.bincount(dst, minlength=N) + 1).astype(np.float32)

    core_of = dst // NPC
    wloc = (dst - core_of * NPC) // WIN
    half_of = (src >= HALF).astype(np.int64)

    keys = (core_of * NWA + wloc) * 2 + half_of
    order = np.argsort(keys, kind="stable")
    s_sorted = src[order]
    d_sorted = dst[order]
    counts = np.bincount(keys[order], minlength=C * NWA * 2).reshape(C, NWA, 2)
    starts = np.zeros(C * NWA * 2 + 1, dtype=np.int64)
    np.cumsum(counts.reshape(-1), out=starts[1:])

    # tiles per (window, half), equalized across cores (single SPMD program)
    nt2 = np.ceil(counts / P).astype(np.int64).max(axis=0)        # [NWA, 2]
    meta = dict(nt=[], tile_base=[], T=[], chunks=[])
    per_core = dict(idx16=[], dst_rel=[])
    ch_cols = CHUNK_TOK // P
    for h in range(2):
        nt = nt2[:, h]
        tile_base = np.zeros(NWA + 1, dtype=np.int64)
        np.cumsum(nt, out=tile_base[1:])
        T = int(tile_base[-1])
        L = T * P
        idx16 = np.zeros((C, 128, max(1, T * 8)), dtype=np.int16)
        dst_rel = np.full((C, P, max(1, T)), -1.0, dtype=np.float32)
        for c in range(C):
            flat_idx = np.zeros(max(16, L), dtype=np.int16)
            for w in range(NWA):
                cnt = int(counts[c, w, h])
                if cnt == 0:
                    continue
                e0 = int(starts[(c * NWA + w) * 2 + h])
                tok = tile_base[w] * P + np.arange(cnt)
                flat_idx[tok] = (s_sorted[e0:e0 + cnt] - h * HALF).astype(
                    np.int16)
                dst_rel[c, tok % P, tok // P] = (
                    d_sorted[e0:e0 + cnt] - c * NPC - w * WIN
                ).astype(np.float32)
            if L > 0:
                wrapped = flat_idx[:L].reshape(L // 16, 16).T      # [16, L/16]
                idx16[c] = np.tile(wrapped, (8, 1))
        meta["nt"].append(nt.tolist())
        meta["tile_base"].append(tile_base.tolist())
        meta["T"].append(T)
        meta["chunks"].append(
            [(c0, min(c0 + ch_cols, T)) for c0 in range(0, T, ch_cols)])
        per_core["idx16"].append(idx16)
        per_core["dst_rel"].append(dst_rel)

    # batch ids per own node, node-major [P, NWC], -1 for tail padding
    batch_col = np.full((C, P, NWC), -1.0, dtype=np.float32)
    for c in range(C):
        own = batch[c * NPC:(c + 1) * NPC].astype(np.float32)
        n = np.arange(NPC)
        batch_col[c, n % P, n // P] = own

    # deg for own nodes in both layouts (pad with 1.0 -> dinv finite)
    deg_col = np.ones((C, P, NWC), dtype=np.float32)
    deg_row = np.ones((C, 1, NWA * WIN), dtype=np.float32)
    for c in range(C):
        own = deg[c * NPC:(c + 1) * NPC]
        n = np.arange(NPC)
        deg_col[c, n % P, n // P] = own
        deg_row[c, 0, :NPC] = own

    meta["half"] = HALF
    per_core.update(batch_col=batch_col, deg_col=deg_col, deg_row=deg_row)
    return meta, per_core


# ---------------------------------------------------------------------------
# Bass program
# ---------------------------------------------------------------------------

def build_program(cfg: Cfg, meta, debug_outputs: bool = False):
    N, C, NPC, G = cfg.n_nodes, cfg.n_cores, cfg.npc, cfg.n_graphs
    NWA, NWC = cfg.nwa, cfg.nwc
    HALF = meta["half"]
    T2 = meta["T"]
    nt2 = meta["nt"]
    tile_base2 = meta["tile_base"]
    chunks2 = meta["chunks"]
    f32 = mybir.dt.float32
    i16 = mybir.dt.int16
    FT = mybir.ActivationFunctionType
    ALU = mybir.AluOpType
    GI = max(G, WIN)                      # iota width for S builds + pooling
    ch_cols = CHUNK_TOK // P

    nc = bacc.Bacc("TRN2", target_bir_lowering=False, debug=False,
                   num_devices=C, num_swdge_queues=4)

    # ---- I/O ----
    xT_d = nc.dram_tensor("xT", [F, NPC], f32, kind="ExternalInput")
    W1_d = nc.dram_tensor("W1", [F, F], f32, kind="ExternalInput")
    W2_d = nc.dram_tensor("W2", [F, F], f32, kind="ExternalInput")
    b1_d = nc.dram_tensor("b1", [F, 1], f32, kind="ExternalInput")
    b2_d = nc.dram_tensor("b2", [F, 1], f32, kind="ExternalInput")
    woutf_d = nc.dram_tensor("wout_f", [F, 1], f32, kind="ExternalInput")
    wlast_d = nc.dram_tensor("wlast", [1, 1], f32, kind="ExternalInput")
    bout_d = nc.dram_tensor("bout", [1, 1], f32, kind="ExternalInput")
    depth_d = nc.dram_tensor("depth_row", [1, G], f32, kind="ExternalInput")
    degc_d = nc.dram_tensor("deg_col", [P, NWC], f32, kind="ExternalInput")
    degr_d = nc.dram_tensor("deg_row", [1, NWA * WIN], f32,
                            kind="ExternalInput")
    iota_d = nc.dram_tensor("iota_all", [P, GI], f32, kind="ExternalInput")
    idx_d = [nc.dram_tensor(f"idx16_{h}", [128, max(1, T2[h] * 8)], i16,
                            kind="ExternalInput") for h in range(2)]
    drel_d = [nc.dram_tensor(f"dst_rel_{h}", [P, max(1, T2[h])], f32,
                             kind="ExternalInput") for h in range(2)]
    bcol_d = nc.dram_tensor("batch_col", [P, NWC], f32, kind="ExternalInput")
    y_d = nc.dram_tensor("y_out", [1, G], f32, kind="ExternalOutput")
    dbg = {}
    if debug_outputs:
        dbg["h1T"] = nc.dram_tensor("dbg_h1T", [F, NPC], f32,
                                    kind="ExternalOutput")
        dbg["h2T"] = nc.dram_tensor("dbg_h2T", [F, NPC], f32,
                                    kind="ExternalOutput")
        dbg["g1"] = nc.dram_tensor("dbg_g1", [N, F], f32,
                                   kind="ExternalOutput")
        dbg["pool"] = nc.dram_tensor("dbg_pool", [2, G], f32,
                                     kind="ExternalOutput")

    with tile.TileContext(nc) as tc:
        with (
            tc.tile_pool(name="const", bufs=1) as const_pool,
            tc.tile_pool(name="big", bufs=1) as big_pool,
            tc.tile_pool(name="gbuf", bufs=4) as gbuf_pool,
            tc.tile_pool(name="work", bufs=2) as work_pool,
            tc.tile_pool(name="spool", bufs=2) as s_pool,
            tc.tile_pool(name="psA", bufs=2, space="PSUM") as psumA,
            tc.tile_pool(name="psB", bufs=2, space="PSUM") as psumB,
            tc.tile_pool(name="psC", bufs=2, space="PSUM") as psumC,
            tc.tile_pool(name="dram", bufs=1, space="DRAM") as dram_pool,
        ):
            # ---- load constants ----
            def load(pool, dram_t, shape, dtype=f32, name=None):
                t = pool.tile(shape, dtype, name=name or dram_t.name + "_sb")
                nc.sync.dma_start(t[:], dram_t[:])
                return t

            xT = load(big_pool, xT_d, [F, NPC])
            W1 = load(const_pool, W1_d, [F, F])
            W2 = load(const_pool, W2_d, [F, F])
            b1 = load(const_pool, b1_d, [F, 1])
            b2 = load(const_pool, b2_d, [F, 1])
            woutf = load(const_pool, woutf_d, [F, 1])
            wlast = load(const_pool, wlast_d, [1, 1])
            bout = load(const_pool, bout_d, [1, 1])
            depth = load(const_pool, depth_d, [1, G])
            deg_col = load(const_pool, degc_d, [P, NWC])
            deg_row = load(const_pool, degr_d, [1, NWA * WIN])
            iota = load(const_pool, iota_d, [P, GI])
            idx_sb = [load(big_pool, idx_d[h], [128, max(1, T2[h] * 8)],
                           i16, name=f"idx_sb{h}") for h in range(2)]
            drel = [load(big_pool, drel_d[h], [P, max(1, T2[h])],
                         name=f"drel_sb{h}") for h in range(2)]
            bcol = load(const_pool, bcol_d, [P, NWC])

            ones1F = const_pool.tile([1, F], f32, name="ones1F")
            nc.vector.memset(ones1F[:], 1.0)
            ident = const_pool.tile([P, P], f32, name="ident")
            make_identity(nc, ident[:])

            # ---- dinv: node-major for the linear phase ----
            dinv_col = const_pool.tile([P, NWC], f32, name="dinv_col")
            nc.scalar.activation(dinv_col[:], deg_col[:], FT.Sqrt)
            nc.vector.reciprocal(dinv_col[:], dinv_col[:])

            # ---- dinv broadcast across features: bcast deg then sqrt+recip
            dinvT = big_pool.tile([F, NWA * WIN], f32, name="dinvT")
            for j0 in range(0, NWA * WIN, 512):
                j1 = min(j0 + 512, NWA * WIN)
                ps = psumB.tile([F, 512], f32, name="bc_ps", tag="psB")
                nc.tensor.matmul(ps[:, : j1 - j0], ones1F[:],
                                 deg_row[:, j0:j1], start=True, stop=True)
                nc.scalar.activation(dinvT[:, j0:j1], ps[:, : j1 - j0],
                                     FT.Sqrt)
                nc.vector.reciprocal(dinvT[:, j0:j1], dinvT[:, j0:j1])

            # ---- internal DRAM gather tables ----
            g1_own = dram_pool.tile([NPC, F], f32, name="g1_own")
            g2_own = dram_pool.tile([NPC, F], f32, name="g2_own")
            g1_full = dram_pool.tile([N, F], f32, name="g1_full",
                                     addr_space="Shared")
            g2_full = dram_pool.tile([N, F], f32, name="g2_full",
                                     addr_space="Shared")
            cc_in = dram_pool.tile([2, G], f32, name="cc_in")
            cc_out = dram_pool.tile([2, G], f32, name="cc_out",
                                    addr_space="Shared")

            # ---- linear phase: g_own = dinv * (h @ W), h given transposed
            def linear_phase(hT, W, g_own_dram):
                for w in range(NWC):
                    n0 = w * P
                    n1 = min(n0 + P, NPC)
                    m = n1 - n0
                    ps = psumA.tile([P, F], f32, name="lin_ps", tag="psA")
                    nc.tensor.matmul(ps[:m, :], hT[:, n0:n1], W[:],
                                     start=True, stop=True)
                    gt = work_pool.tile([P, F], f32, name="lin_g")
                    nc.vector.tensor_scalar(gt[:m, :], ps[:m, :],
                                            dinv_col[:m, w:w + 1], None,
                                            op0=ALU.mult)
                    nc.sync.dma_start(g_own_dram[n0:n1, :], gt[:m, :])

            def allgather(g_own, g_full):
                nc.gpsimd.collective_compute(
                    "AllGather", ALU.bypass,
                    replica_groups=[list(range(C))],
                    ins=[g_own.opt()], outs=[g_full.opt()],
                )

            # ---- pooling window op (layer 2 only), per 128-node subwindow
            pool_ps = psumC.tile([2, G], f32, name="pool_ps", tag="psC")

            def pool_sub(h2T, s):
                n0 = s * P
                m = min(P, NPC - n0)
                sc_ps = psumC.tile([P, 1], f32, name="score_ps", tag="psD")
                nc.tensor.matmul(sc_ps[:m, :], h2T[:, n0:n0 + m], woutf[:],
                                 start=True, stop=True)
                sc = work_pool.tile([P, 2], f32, name="score_sb")
                nc.vector.memset(sc[:], 0.0)
                nc.vector.memset(sc[:m, 1:2], 1.0)
                nc.vector.tensor_copy(sc[:m, 0:1], sc_ps[:m, :])
                sg = work_pool.tile([P, G], f32, name="sel_pool")
                nc.vector.tensor_scalar(sg[:], iota[:, :G], bcol[:, s:s + 1],
                                        None, op0=ALU.is_equal)
                nc.tensor.matmul(pool_ps[:], sc[:], sg[:],
                                 start=(s == 0), stop=(s == NWC - 1))

            # ---- aggregation: hT = relu(dinv * (scatter_add(g[src]) + g_own)
            #      + b), where the g_own term is the self-loop contribution.
            qrr = [0]

            def agg_phase(g_full, g_own, b_tile, hT_out, suffix, do_pool):
                tabs = [g_full[0:HALF, :], g_full[HALF:N, :]]
                chunk_tiles = {}

                def ensure_chunk(h, ci):
                    key = (h, ci)
                    if key in chunk_tiles:
                        return chunk_tiles[key]
                    c0, c1 = chunks2[h][ci]
                    ntok = (c1 - c0) * P
                    ct = gbuf_pool.tile([P, ch_cols * F], f32,
                                        name="chunk_" + suffix, tag="chunk")
                    nc.gpsimd.dma_gather(
                        out_ap=ct[:, :(c1 - c0) * F].rearrange(
                            "p (s e) -> p s e", e=F),
                        in_ap=tabs[h],
                        idxs_ap=idx_sb[h][:, c0 * 8:c1 * 8],
                        num_idxs=ntok,
                        num_idxs_reg=ntok,
                        elem_size=F,
                        single_packet=False,
                        queue_num=qrr[0] % 4,
                    )
                    qrr[0] += 1
                    chunk_tiles[key] = (ct, c0)
                    return chunk_tiles[key]

                for w in range(NWA):
                    n0 = w * WIN
                    m = min(WIN, NPC - n0)
                    n_tr = (m + P - 1) // P
                    ps = psumA.tile([F, WIN], f32, name="agg_ps_" + suffix,
                                    tag="psA")
                    nmm = nt2[0][w] + nt2[1][w] + n_tr
                    i = 0
                    for h in range(2):
                        ntw = nt2[h][w]
                        base = tile_base2[h][w]
                        for b0 in range(0, ntw, SBATCH):
                            nb = min(SBATCH, ntw - b0)
                            sw = s_pool.tile([P, nb * WIN], f32,
                                             name="sel_" + suffix, tag="sel",
                                             padded_shape=[P, SBATCH * WIN])
                            nc.vector.tensor_tensor(
                                sw[:].rearrange("p (t j) -> p t j", j=WIN),
                                iota[:, :WIN].rearrange(
                                    "p (o j) -> p o j", o=1).to_broadcast(
                                        (P, nb, WIN)),
                                drel[h][:, base + b0:base + b0 + nb].rearrange(
                                    "p (t o) -> p t o", o=1).to_broadcast(
                                        (P, nb, WIN)),
                                op=ALU.is_equal)
                            for t in range(nb):
                                gt = base + b0 + t
                                ct, c0 = ensure_chunk(h, gt // ch_cols)
                                col = gt - c0
                                nc.tensor.matmul(
                                    ps[:], ct[:, col * F:(col + 1) * F],
                                    sw[:, t * WIN:(t + 1) * WIN],
                                    start=(i == 0), stop=False)
                                i += 1
                    # self-loop term: transpose own g rows into the window
                    for s in range(n_tr):
                        ms = min(P, m - s * P)
                        gsl = work_pool.tile([P, F], f32,
                                             name="gself_" + suffix)
                        nc.sync.dma_start(
                            gsl[:ms, :], g_own[n0 + s * P:n0 + s * P + ms, :])
                        i += 1
                        nc.tensor.matmul(
                            ps[:, s * P:s * P + ms], gsl[:ms, :],
                            ident[:ms, :ms], is_transpose=True,
                            start=False, stop=(i == nmm))
                    tmp = work_pool.tile([F, WIN], f32, name="fin_" + suffix)
                    nc.vector.tensor_tensor(
                        tmp[:, :m], ps[:, :m], dinvT[:, n0:n0 + m],
                        op=ALU.mult)
                    nc.scalar.activation(hT_out[:, n0:n0 + m], tmp[:, :m],
                                         FT.Relu, bias=b_tile[:])
                    if do_pool:
                        for s in range(n0 // P, (n0 + m + P - 1) // P):
                            pool_sub(hT_out, s)

            h1T = big_pool.tile([F, NPC], f32, name="h1T")
            h2T = big_pool.tile([F, NPC], f32, name="h2T", tag="xT_sb")

            # ---- layer 1 ----
            linear_phase(xT, W1, g1_own)
            allgather(g1_own, g1_full)
            agg_phase(g1_full, g1_own, b1, h1T, "l1", do_pool=False)

            # ---- layer 2 (pooling fused into the window loop) ----
            linear_phase(h1T, W2, g2_own)
            allgather(g2_own, g2_full)
            agg_phase(g2_full, g2_own, b2, h2T, "l2", do_pool=True)

            # ---- pooled sums/counts AllReduce + head ----
            pool_sb = const_pool.tile([2, G], f32, name="pool_sb")
            nc.vector.tensor_copy(pool_sb[:], pool_ps[:])
            nc.sync.dma_start(cc_in[:], pool_sb[:])
            nc.gpsimd.collective_compute(
                "AllReduce", ALU.add, replica_groups=[list(range(C))],
                ins=[cc_in.opt()], outs=[cc_out.opt()])
            pool_g0 = const_pool.tile([1, G], f32, name="pool_g0")
            pool_g1 = const_pool.tile([1, G], f32, name="pool_g1")
            nc.sync.dma_start(pool_g0[:], cc_out[0:1, :])
            nc.sync.dma_start(pool_g1[:], cc_out[1:2, :])
            if debug_outputs:
                nc.sync.dma_start(dbg["pool"][:], cc_out[:])

            # y = sums/max(cnt,1) + depth*wlast + bout
            cnt = const_pool.tile([1, G], f32, name="cnt_row")
            nc.vector.tensor_scalar(cnt[:], pool_g1[:], 1.0, None,
                                    op0=ALU.max)
            nc.vector.reciprocal(cnt[:], cnt[:])
            y = const_pool.tile([1, G], f32, name="y_row")
            nc.vector.tensor_tensor(y[:], pool_g0[:], cnt[:], op=ALU.mult)
            dterm = const_pool.tile([1, G], f32, name="dterm")
            nc.vector.tensor_scalar(dterm[:], depth[:], wlast[:], None,
                                    op0=ALU.mult)
            nc.vector.tensor_tensor(y[:], y[:], dterm[:], op=ALU.add)
            nc.vector.tensor_scalar(y[:], y[:], bout[:], None, op0=ALU.add)
            nc.sync.dma_start(y_d[:], y[:])

            if debug_outputs:
                nc.sync.dma_start(dbg["h1T"][:], h1T[:])
                nc.sync.dma_start(dbg["h2T"][:], h2T[:])
                nc.gpsimd.dma_start(dbg["g1"][:], g1_full[:])

    nc.compile()
    return nc


# ---------------------------------------------------------------------------
# full pipeline
# ---------------------------------------------------------------------------

def make_in_maps(cfg: Cfg, meta, per_core, x, depth, W1, b1, W2, b2, Wout,
                 bout):
    C, NPC, G = cfg.n_cores, cfg.npc, cfg.n_graphs
    GI = max(G, WIN)
    iota = np.broadcast_to(np.arange(GI, dtype=np.float32), (P, GI)).copy()
    in_maps = []
    for c in range(C):
        xT = np.ascontiguousarray(x[c * NPC:(c + 1) * NPC, :].T)
        in_maps.append({
            "xT": xT,
            "W1": np.ascontiguousarray(W1),
            "W2": np.ascontiguousarray(W2),
            "b1": b1.reshape(F, 1).copy(),
            "b2": b2.reshape(F, 1).copy(),
            "wout_f": Wout[:F, :].copy(),
            "wlast": Wout[F:, :].copy(),
            "bout": bout.reshape(1, 1).copy(),
            "depth_row": depth.reshape(1, G).copy(),
            "deg_col": per_core["deg_col"][c],
            "deg_row": per_core["deg_row"][c],
            "iota_all": iota,
            "idx16_0": per_core["idx16"][0][c],
            "idx16_1": per_core["idx16"][1][c],
            "dst_rel_0": per_core["dst_rel"][0][c],
            "dst_rel_1": per_core["dst_rel"][1][c],
            "batch_col": per_core["batch_col"][c],
        })
    return in_maps


def kernel(x, edge_index, batch, depth, W1, b1, W2, b2, Wout, bout):
    cfg = Cfg()
    x = np.asarray(x, dtype=np.float32)
    edge_index = np.asarray(edge_index)
    batch = np.asarray(batch)
    depth = np.asarray(depth, dtype=np.float32)
    W1 = np.asarray(W1, dtype=np.float32)
    b1 = np.asarray(b1, dtype=np.float32)
    W2 = np.asarray(W2, dtype=np.float32)
    b2 = np.asarray(b2, dtype=np.float32)
    Wout = np.asarray(Wout, dtype=np.float32)
    bout = np.asarray(bout, dtype=np.float32)

    meta, per_core = host_prep(cfg, edge_index, batch)
    nc = build_program(cfg, meta)
    in_maps = make_in_maps(cfg, meta, per_core, x, depth, W1, b1, W2, b2,
                           Wout, bout)
    from concourse import bass_utils
    res = bass_utils.run_bass_kernel_spmd(
        nc, in_maps, core_ids=list(range(cfg.n_cores)))
    y = np.asarray(res.results[0]["y_out"]).reshape(cfg.n_graphs)
    return y.astype(np.float32)


if __name__ == "__main__":
    sys.path.insert(0, os.path.dirname(os.path.abspath(__file__)))
    import reference
    inputs = {k: np.asarray(v) for k, v in reference.setup_inputs().items()}
    out = kernel(**inputs)
    print("kernel output:", out[:8])


# revision 12
# speedup vs baseline: 1.4939x; 1.2502x over previous
"""Distributed GCN (2x GCNConv + global_mean_pool + linear head) on 8 Trainium2
NeuronCores via Bass/Tile.

Sharding: nodes are split into 8 contiguous ranges; each core owns the edges
whose *destination* falls in its range.  Weights are replicated.  Per layer
each core computes g = dinv * (h @ W) for its own node slice, the slices are
AllGathered into a full gather table in HBM, the core then gathers g[src] for
its edges with dma_gather (two <=32768-row table halves, int16 indices) and
reduces them per 256-node dst window with one-hot-matrix matmuls accumulated in
PSUM (segmented scatter-add as matmul).  Self-loop terms enter the same PSUM
accumulation as PE transposes of the core's own g rows.  Pooled sums/counts
are AllReduced at the end.
"""

import math
import os
import sys

import numpy as np

for _p in ("/opt/trn_rl_repo", "/root/.axon_site/_ro/trn_rl_repo"):
    if os.path.isdir(_p) and _p not in sys.path:
        sys.path.append(_p)

import concourse.bacc as bacc
import concourse.bass as bass
import concourse.tile as tile
from concourse import mybir
from concourse.masks import make_identity

F = 64            # feature/hidden width
P = 128           # partitions
WIN = 256         # dst-window (PSUM segment) size in nodes
CHUNK_TOK = 2048  # gather tokens per dma_gather call
SBATCH = 4        # selection-matrix tiles built per DVE op


class Cfg:
    def __init__(self, n_nodes=50000, n_edges=800000, n_graphs=512, n_cores=8):
        assert n_nodes % n_cores == 0
        self.n_nodes = n_nodes
        self.n_edges = n_edges
        self.n_graphs = n_graphs
        self.n_cores = n_cores
        self.npc = n_nodes // n_cores             # nodes per core
        self.nwa = math.ceil(self.npc / WIN)      # agg windows per core
        self.nwc = math.ceil(self.npc / P)        # 128-col windows per core


# ---------------------------------------------------------------------------
# host-side graph partitioning (integer/structural work only)
# ---------------------------------------------------------------------------

def host_prep(cfg: Cfg, edge_index: np.ndarray, batch: np.ndarray):
    N, C, NPC, NWA = cfg.n_nodes, cfg.n_cores, cfg.npc, cfg.nwa
    NWC = cfg.nwc
    HALF = N // 2
    assert HALF <= 32768 and N - HALF <= 32768
    src = edge_index[0].astype(np.int64)
    dst = edge_index[1].astype(np.int64)

    # reference degree includes the self-loop; the self-loop term itself is
    # applied on-device as a transpose of the core's own g rows.
    deg = (np.bincount(dst, minlength=N) + 1).astype(np.float32)

    core_of = dst // NPC
    wloc = (dst - core_of * NPC) // WIN
    half_of = (src >= HALF).astype(np.int64)

    keys = (core_of * NWA + wloc) * 2 + half_of
    order = np.argsort(keys, kind="stable")
    s_sorted = src[order]
    d_sorted = dst[order]
    counts = np.bincount(keys[order], minlength=C * NWA * 2).reshape(C, NWA, 2)
    starts = np.zeros(C * NWA * 2 + 1, dtype=np.int64)
    np.cumsum(counts.reshape(-1), out=starts[1:])

    # tiles per (window, half), equalized across cores (single SPMD program)
    nt2 = np.ceil(counts / P).astype(np.int64).max(axis=0)        # [NWA, 2]
    meta = dict(nt=[], tile_base=[], T=[], chunks=[])
    per_core = dict(idx16=[], dst_rel=[])
    ch_cols = CHUNK_TOK // P
    for h in range(2):
        nt = nt2[:, h]
        tile_base = np.zeros(NWA + 1, dtype=np.int64)
        np.cumsum(nt, out=tile_base[1:])
        T = int(tile_base[-1])
        L = T * P
        idx16 = np.zeros((C, 128, max(1, T * 8)), dtype=np.int16)
        dst_rel = np.full((C, P, max(1, T)), -1.0, dtype=np.float32)
        for c in range(C):
            flat_idx = np.zeros(max(16, L), dtype=np.int16)
            for w in range(NWA):
                cnt = int(counts[c, w, h])
                if cnt == 0:
                    continue
                e0 = int(starts[(c * NWA + w) * 2 + h])
                tok = tile_base[w] * P + np.arange(cnt)
                flat_idx[tok] = (s_sorted[e0:e0 + cnt] - h * HALF).astype(
                    np.int16)
                dst_rel[c, tok % P, tok // P] = (
                    d_sorted[e0:e0 + cnt] - c * NPC - w * WIN
                ).astype(np.float32)
            if L > 0:
                wrapped = flat_idx[:L].reshape(L // 16, 16).T      # [16, L/16]
                idx16[c] = np.tile(wrapped, (8, 1))
        meta["nt"].append(nt.tolist())
        meta["tile_base"].append(tile_base.tolist())
        meta["T"].append(T)
        meta["chunks"].append(
            [(c0, min(c0 + ch_cols, T)) for c0 in range(0, T, ch_cols)])
        per_core["idx16"].append(idx16)
        per_core["dst_rel"].append(dst_rel)

    # batch ids per own node, node-major [P, NWC], -1 for tail padding
    batch_col = np.full((C, P, NWC), -1.0, dtype=np.float32)
    for c in range(C):
        own = batch[c * NPC:(c + 1) * NPC].astype(np.float32)
        n = np.arange(NPC)
        batch_col[c, n % P, n // P] = own

    # deg for own nodes in both layouts (pad with 1.0 -> dinv finite)
    deg_col = np.ones((C, P, NWC), dtype=np.float32)
    deg_row = np.ones((C, 1, NWA * WIN), dtype=np.float32)
    for c in range(C):
        own = deg[c * NPC:(c + 1) * NPC]
        n = np.arange(NPC)
        deg_col[c, n % P, n // P] = own
        deg_row[c, 0, :NPC] = own

    meta["half"] = HALF
    per_core.update(batch_col=batch_col, deg_col=deg_col, deg_row=deg_row)
    return meta, per_core


# ---------------------------------------------------------------------------
# Bass program
# ---------------------------------------------------------------------------

def build_program(cfg: Cfg, meta, debug_outputs: bool = False):
    N, C, NPC, G = cfg.n_nodes, cfg.n_cores, cfg.npc, cfg.n_graphs
    NWA, NWC = cfg.nwa, cfg.nwc
    HALF = meta["half"]
    T2 = meta["T"]
    nt2 = meta["nt"]
    tile_base2 = meta["tile_base"]
    chunks2 = meta["chunks"]
    f32 = mybir.dt.float32
    i16 = mybir.dt.int16
    FT = mybir.ActivationFunctionType
    ALU = mybir.AluOpType
    GI = max(G, WIN)                      # iota width for S builds + pooling
    ch_cols = CHUNK_TOK // P

    nc = bacc.Bacc("TRN2", target_bir_lowering=False, debug=False,
                   num_devices=C, num_swdge_queues=4)

    # ---- I/O ----
    xT_d = nc.dram_tensor("xT", [F, NPC], f32, kind="ExternalInput")
    W1_d = nc.dram_tensor("W1", [F, F], f32, kind="ExternalInput")
    W2_d = nc.dram_tensor("W2", [F, F], f32, kind="ExternalInput")
    b1_d = nc.dram_tensor("b1", [F, 1], f32, kind="ExternalInput")
    b2_d = nc.dram_tensor("b2", [F, 1], f32, kind="ExternalInput")
    woutf_d = nc.dram_tensor("wout_f", [F, 1], f32, kind="ExternalInput")
    wlast_d = nc.dram_tensor("wlast", [1, 1], f32, kind="ExternalInput")
    bout_d = nc.dram_tensor("bout", [1, 1], f32, kind="ExternalInput")
    depth_d = nc.dram_tensor("depth_row", [1, G], f32, kind="ExternalInput")
    degc_d = nc.dram_tensor("deg_col", [P, NWC], f32, kind="ExternalInput")
    degr_d = nc.dram_tensor("deg_row", [1, NWA * WIN], f32,
                            kind="ExternalInput")
    iota_d = nc.dram_tensor("iota_all", [P, GI], f32, kind="ExternalInput")
    idx_d = [nc.dram_tensor(f"idx16_{h}", [128, max(1, T2[h] * 8)], i16,
                            kind="ExternalInput") for h in range(2)]
    drel_d = [nc.dram_tensor(f"dst_rel_{h}", [P, max(1, T2[h])], f32,
                             kind="ExternalInput") for h in range(2)]
    bcol_d = nc.dram_tensor("batch_col", [P, NWC], f32, kind="ExternalInput")
    y_d = nc.dram_tensor("y_out", [1, G], f32, kind="ExternalOutput")
    dbg = {}
    if debug_outputs:
        dbg["h1T"] = nc.dram_tensor("dbg_h1T", [F, NPC], f32,
                                    kind="ExternalOutput")
        dbg["h2T"] = nc.dram_tensor("dbg_h2T", [F, NPC], f32,
                                    kind="ExternalOutput")
        dbg["g1"] = nc.dram_tensor("dbg_g1", [N, F], f32,
                                   kind="ExternalOutput")
        dbg["pool"] = nc.dram_tensor("dbg_pool", [2, G], f32,
                                     kind="ExternalOutput")

    with tile.TileContext(nc) as tc:
        with (
            tc.tile_pool(name="const", bufs=1) as const_pool,
            tc.tile_pool(name="big", bufs=1) as big_pool,
            tc.tile_pool(name="gbuf", bufs=8) as gbuf_pool,
            tc.tile_pool(name="work", bufs=2) as work_pool,
            tc.tile_pool(name="spool", bufs=3) as s_pool,
            tc.tile_pool(name="psA", bufs=3, space="PSUM") as psumA,
            tc.tile_pool(name="psB", bufs=2, space="PSUM") as psumB,
            tc.tile_pool(name="psC", bufs=1, space="PSUM") as psumC,
            tc.tile_pool(name="dram", bufs=1, space="DRAM") as dram_pool,
        ):
            # ---- load constants ----
            def load(pool, dram_t, shape, dtype=f32, name=None):
                t = pool.tile(shape, dtype, name=name or dram_t.name + "_sb")
                nc.sync.dma_start(t[:], dram_t[:])
                return t

            xT = load(big_pool, xT_d, [F, NPC])
            W1 = load(const_pool, W1_d, [F, F])
            W2 = load(const_pool, W2_d, [F, F])
            b1 = load(const_pool, b1_d, [F, 1])
            b2 = load(const_pool, b2_d, [F, 1])
            woutf = load(const_pool, woutf_d, [F, 1])
            wlast = load(const_pool, wlast_d, [1, 1])
            bout = load(const_pool, bout_d, [1, 1])
            depth = load(const_pool, depth_d, [1, G])
            deg_col = load(const_pool, degc_d, [P, NWC])
            deg_row = load(const_pool, degr_d, [1, NWA * WIN])
            iota = load(const_pool, iota_d, [P, GI])
            idx_sb = [load(big_pool, idx_d[h], [128, max(1, T2[h] * 8)],
                           i16, name=f"idx_sb{h}") for h in range(2)]
            drel = [load(big_pool, drel_d[h], [P, max(1, T2[h])],
                         name=f"drel_sb{h}") for h in range(2)]
            bcol = load(const_pool, bcol_d, [P, NWC])

            ones1F = const_pool.tile([1, F], f32, name="ones1F")
            nc.vector.memset(ones1F[:], 1.0)
            ident = const_pool.tile([P, P], f32, name="ident")
            make_identity(nc, ident[:])

            # ---- dinv: node-major for the linear phase ----
            dinv_col = const_pool.tile([P, NWC], f32, name="dinv_col")
            nc.scalar.activation(dinv_col[:], deg_col[:], FT.Sqrt)
            nc.vector.reciprocal(dinv_col[:], dinv_col[:])

            # ---- dinv broadcast across features: bcast deg then sqrt+recip
            dinvT = big_pool.tile([F, NWA * WIN], f32, name="dinvT")
            for j0 in range(0, NWA * WIN, 512):
                j1 = min(j0 + 512, NWA * WIN)
                ps = psumB.tile([F, 512], f32, name="bc_ps", tag="psB")
                nc.tensor.matmul(ps[:, : j1 - j0], ones1F[:],
                                 deg_row[:, j0:j1], start=True, stop=True)
                nc.scalar.activation(dinvT[:, j0:j1], ps[:, : j1 - j0],
                                     FT.Sqrt)
                nc.vector.reciprocal(dinvT[:, j0:j1], dinvT[:, j0:j1])

            # ---- internal DRAM gather tables ----
            g1_own = dram_pool.tile([NPC, F], f32, name="g1_own")
            g2_own = dram_pool.tile([NPC, F], f32, name="g2_own")
            g1_full = dram_pool.tile([N, F], f32, name="g1_full",
                                     addr_space="Shared")
            g2_full = dram_pool.tile([N, F], f32, name="g2_full",
                                     addr_space="Shared")
            cc_in = dram_pool.tile([2, G], f32, name="cc_in")
            cc_out = dram_pool.tile([2, G], f32, name="cc_out",
                                    addr_space="Shared")

            # ---- linear phase: g_own = dinv * (h @ W), h given transposed
            def linear_phase(hT, W, g_own_dram):
                for w in range(NWC):
                    n0 = w * P
                    n1 = min(n0 + P, NPC)
                    m = n1 - n0
                    ps = psumA.tile([P, F], f32, name="lin_ps", tag="psA")
                    nc.tensor.matmul(ps[:m, :], hT[:, n0:n1], W[:],
                                     start=True, stop=True)
                    gt = work_pool.tile([P, F], f32, name="lin_g")
                    nc.vector.tensor_scalar(gt[:m, :], ps[:m, :],
                                            dinv_col[:m, w:w + 1], None,
                                            op0=ALU.mult)
                    nc.sync.dma_start(g_own_dram[n0:n1, :], gt[:m, :])

            def allgather(g_own, g_full):
                nc.gpsimd.collective_compute(
                    "AllGather", ALU.bypass,
                    replica_groups=[list(range(C))],
                    ins=[g_own.opt()], outs=[g_full.opt()],
                )

            # ---- pooling window op (layer 2 only), per 128-node subwindow
            pool_ps = psumC.tile([2, G], f32, name="pool_ps", tag="psC")

            def pool_sub(h2T, s):
                n0 = s * P
                m = min(P, NPC - n0)
                sc_ps = psumB.tile([P, 1], f32, name="score_ps", tag="psB")
                nc.tensor.matmul(sc_ps[:m, :], h2T[:, n0:n0 + m], woutf[:],
                                 start=True, stop=True)
                sc = work_pool.tile([P, 2], f32, name="score_sb")
                nc.vector.memset(sc[:], 0.0)
                nc.vector.memset(sc[:m, 1:2], 1.0)
                nc.vector.tensor_copy(sc[:m, 0:1], sc_ps[:m, :])
                sg = work_pool.tile([P, G], f32, name="sel_pool")
                nc.vector.tensor_scalar(sg[:], iota[:, :G], bcol[:, s:s + 1],
                                        None, op0=ALU.is_equal)
                nc.tensor.matmul(pool_ps[:], sc[:], sg[:],
                                 start=(s == 0), stop=(s == NWC - 1))

            # ---- aggregation: hT = relu(dinv * (scatter_add(g[src]) + g_own)
            #      + b), where the g_own term is the self-loop contribution.
            qrr = [0]

            def agg_phase(g_full, g_own, b_tile, hT_out, suffix, do_pool):
                tabs = [g_full[0:HALF, :], g_full[HALF:N, :]]
                chunk_tiles = {}

                def ensure_chunk(h, ci):
                    key = (h, ci)
                    if key in chunk_tiles:
                        return chunk_tiles[key]
                    c0, c1 = chunks2[h][ci]
                    ntok = (c1 - c0) * P
                    ct = gbuf_pool.tile([P, ch_cols * F], f32,
                                        name="chunk_" + suffix, tag="chunk")
                    nc.gpsimd.dma_gather(
                        out_ap=ct[:, :(c1 - c0) * F].rearrange(
                            "p (s e) -> p s e", e=F),
                        in_ap=tabs[h],
                        idxs_ap=idx_sb[h][:, c0 * 8:c1 * 8],
                        num_idxs=ntok,
                        num_idxs_reg=ntok,
                        elem_size=F,
                        single_packet=False,
                        queue_num=qrr[0] % 4,
                    )
                    qrr[0] += 1
                    chunk_tiles[key] = (ct, c0)
                    return chunk_tiles[key]

                for w in range(NWA):
                    n0 = w * WIN
                    m = min(WIN, NPC - n0)
                    n_tr = (m + P - 1) // P
                    ps = psumA.tile([F, WIN], f32, name="agg_ps_" + suffix,
                                    tag="psA")
                    nmm = nt2[0][w] + nt2[1][w] + n_tr
                    i = 0
                    for h in range(2):
                        ntw = nt2[h][w]
                        base = tile_base2[h][w]
                        for b0 in range(0, ntw, SBATCH):
                            nb = min(SBATCH, ntw - b0)
                            sw = s_pool.tile([P, nb * WIN], f32,
                                             name="sel_" + suffix, tag="sel",
                                             padded_shape=[P, SBATCH * WIN])
                            nc.vector.tensor_tensor(
                                sw[:].rearrange("p (t j) -> p t j", j=WIN),
                                iota[:, :WIN].rearrange(
                                    "p (o j) -> p o j", o=1).to_broadcast(
                                        (P, nb, WIN)),
                                drel[h][:, base + b0:base + b0 + nb].rearrange(
                                    "p (t o) -> p t o", o=1).to_broadcast(
                                        (P, nb, WIN)),
                                op=ALU.is_equal)
                            for t in range(nb):
                                gt = base + b0 + t
                                ct, c0 = ensure_chunk(h, gt // ch_cols)
                                col = gt - c0
                                nc.tensor.matmul(
                                    ps[:], ct[:, col * F:(col + 1) * F],
                                    sw[:, t * WIN:(t + 1) * WIN],
                                    start=(i == 0), stop=False)
                                i += 1
                    # self-loop term: transpose own g rows into the window
                    for s in range(n_tr):
                        ms = min(P, m - s * P)
                        gsl = work_pool.tile([P, F], f32,
                                             name="gself_" + suffix)
                        nc.sync.dma_start(
                            gsl[:ms, :], g_own[n0 + s * P:n0 + s * P + ms, :])
                        i += 1
                        nc.tensor.matmul(
                            ps[:, s * P:s * P + ms], gsl[:ms, :],
                            ident[:ms, :ms], is_transpose=True,
                            start=False, stop=(i == nmm))
                    tmp = work_pool.tile([F, WIN], f32, name="fin_" + suffix)
                    nc.vector.tensor_tensor(
                        tmp[:, :m], ps[:, :m], dinvT[:, n0:n0 + m],
                        op=ALU.mult)
                    nc.scalar.activation(hT_out[:, n0:n0 + m], tmp[:, :m],
                                         FT.Relu, bias=b_tile[:])
                    if do_pool:
                        for s in range(n0 // P, (n0 + m + P - 1) // P):
                            pool_sub(hT_out, s)

            h1T = big_pool.tile([F, NPC], f32, name="h1T")
            h2T = big_pool.tile([F, NPC], f32, name="h2T", tag="xT_sb")

            # ---- layer 1 ----
            linear_phase(xT, W1, g1_own)
            allgather(g1_own, g1_full)
            agg_phase(g1_full, g1_own, b1, h1T, "l1", do_pool=False)

            # ---- layer 2 (pooling fused into the window loop) ----
            linear_phase(h1T, W2, g2_own)
            allgather(g2_own, g2_full)
            agg_phase(g2_full, g2_own, b2, h2T, "l2", do_pool=True)

            # ---- pooled sums/counts AllReduce + head ----
            pool_sb = const_pool.tile([2, G], f32, name="pool_sb")
            nc.vector.tensor_copy(pool_sb[:], pool_ps[:])
            nc.sync.dma_start(cc_in[:], pool_sb[:])
            nc.gpsimd.collective_compute(
                "AllReduce", ALU.add, replica_groups=[list(range(C))],
                ins=[cc_in.opt()], outs=[cc_out.opt()])
            pool_g0 = const_pool.tile([1, G], f32, name="pool_g0")
            pool_g1 = const_pool.tile([1, G], f32, name="pool_g1")
            nc.sync.dma_start(pool_g0[:], cc_out[0:1, :])
            nc.sync.dma_start(pool_g1[:], cc_out[1:2, :])
            if debug_outputs:
                nc.sync.dma_start(dbg["pool"][:], cc_out[:])

            # y = sums/max(cnt,1) + depth*wlast + bout
            cnt = const_pool.tile([1, G], f32, name="cnt_row")
            nc.vector.tensor_scalar(cnt[:], pool_g1[:], 1.0, None,
                                    op0=ALU.max)
            nc.vector.reciprocal(cnt[:], cnt[:])
            y = const_pool.tile([1, G], f32, name="y_row")
            nc.vector.tensor_tensor(y[:], pool_g0[:], cnt[:], op=ALU.mult)
            dterm = const_pool.tile([1, G], f32, name="dterm")
            nc.vector.tensor_scalar(dterm[:], depth[:], wlast[:], None,
                                    op0=ALU.mult)
            nc.vector.tensor_tensor(y[:], y[:], dterm[:], op=ALU.add)
            nc.vector.tensor_scalar(y[:], y[:], bout[:], None, op0=ALU.add)
            nc.sync.dma_start(y_d[:], y[:])

            if debug_outputs:
                nc.sync.dma_start(dbg["h1T"][:], h1T[:])
                nc.sync.dma_start(dbg["h2T"][:], h2T[:])
                nc.gpsimd.dma_start(dbg["g1"][:], g1_full[:])

    nc.compile()
    return nc


# ---------------------------------------------------------------------------
# full pipeline
# ---------------------------------------------------------------------------

def make_in_maps(cfg: Cfg, meta, per_core, x, depth, W1, b1, W2, b2, Wout,
                 bout):
    C, NPC, G = cfg.n_cores, cfg.npc, cfg.n_graphs
    GI = max(G, WIN)
    iota = np.broadcast_to(np.arange(GI, dtype=np.float32), (P, GI)).copy()
    in_maps = []
    for c in range(C):
        xT = np.ascontiguousarray(x[c * NPC:(c + 1) * NPC, :].T)
        in_maps.append({
            "xT": xT,
            "W1": np.ascontiguousarray(W1),
            "W2": np.ascontiguousarray(W2),
            "b1": b1.reshape(F, 1).copy(),
            "b2": b2.reshape(F, 1).copy(),
            "wout_f": Wout[:F, :].copy(),
            "wlast": Wout[F:, :].copy(),
            "bout": bout.reshape(1, 1).copy(),
            "depth_row": depth.reshape(1, G).copy(),
            "deg_col": per_core["deg_col"][c],
            "deg_row": per_core["deg_row"][c],
            "iota_all": iota,
            "idx16_0": per_core["idx16"][0][c],
            "idx16_1": per_core["idx16"][1][c],
            "dst_rel_0": per_core["dst_rel"][0][c],
            "dst_rel_1": per_core["dst_rel"][1][c],
            "batch_col": per_core["batch_col"][c],
        })
    return in_maps


def kernel(x, edge_index, batch, depth, W1, b1, W2, b2, Wout, bout):
    cfg = Cfg()
    x = np.asarray(x, dtype=np.float32)
    edge_index = np.asarray(edge_index)
    batch = np.asarray(batch)
    depth = np.asarray(depth, dtype=np.float32)
    W1 = np.asarray(W1, dtype=np.float32)
    b1 = np.asarray(b1, dtype=np.float32)
    W2 = np.asarray(W2, dtype=np.float32)
    b2 = np.asarray(b2, dtype=np.float32)
    Wout = np.asarray(Wout, dtype=np.float32)
    bout = np.asarray(bout, dtype=np.float32)

    meta, per_core = host_prep(cfg, edge_index, batch)
    nc = build_program(cfg, meta)
    in_maps = make_in_maps(cfg, meta, per_core, x, depth, W1, b1, W2, b2,
                           Wout, bout)
    from concourse import bass_utils
    res = bass_utils.run_bass_kernel_spmd(
        nc, in_maps, core_ids=list(range(cfg.n_cores)))
    y = np.asarray(res.results[0]["y_out"]).reshape(cfg.n_graphs)
    return y.astype(np.float32)


if __name__ == "__main__":
    sys.path.insert(0, os.path.dirname(os.path.abspath(__file__)))
    import reference
    inputs = {k: np.asarray(v) for k, v in reference.setup_inputs().items()}
    out = kernel(**inputs)
    print("kernel output:", out[:8])


# revision 13
# speedup vs baseline: 1.8993x; 1.2713x over previous
"""Distributed GCN (2x GCNConv + global_mean_pool + linear head) on 8 Trainium2
NeuronCores via Bass/Tile.

Sharding: nodes are split into 8 contiguous ranges; each core owns the edges
whose *destination* falls in its range.  Weights are replicated.  Per layer
each core computes g = dinv * (h @ W) for its own node slice, the slices are
AllGathered into a full gather table in HBM, the core then gathers g[src] for
its edges with dma_gather (two <=32768-row table halves, int16 indices) and
reduces them per 256-node dst window with one-hot-matrix matmuls accumulated in
PSUM (segmented scatter-add as matmul).  Self-loop terms enter the same PSUM
accumulation as PE transposes of the core's own g rows.  Pooled sums/counts
are AllReduced at the end.
"""

import math
import os
import sys

import numpy as np

for _p in ("/opt/trn_rl_repo", "/root/.axon_site/_ro/trn_rl_repo"):
    if os.path.isdir(_p) and _p not in sys.path:
        sys.path.append(_p)

import concourse.bacc as bacc
import concourse.bass as bass
import concourse.tile as tile
from concourse import mybir
from concourse.masks import make_identity

F = 64            # feature/hidden width
P = 128           # partitions
WIN = 128         # dst-window (PSUM segment) size in nodes
CHUNK_TOK = 2048  # gather tokens per dma_gather call
SBATCH = 8        # selection-matrix tiles built per DVE op


class Cfg:
    def __init__(self, n_nodes=50000, n_edges=800000, n_graphs=512, n_cores=8):
        assert n_nodes % n_cores == 0
        self.n_nodes = n_nodes
        self.n_edges = n_edges
        self.n_graphs = n_graphs
        self.n_cores = n_cores
        self.npc = n_nodes // n_cores             # nodes per core
        self.nwa = math.ceil(self.npc / WIN)      # agg windows per core
        self.nwc = math.ceil(self.npc / P)        # 128-col windows per core


# ---------------------------------------------------------------------------
# host-side graph partitioning (integer/structural work only)
# ---------------------------------------------------------------------------

def host_prep(cfg: Cfg, edge_index: np.ndarray, batch: np.ndarray):
    N, C, NPC, NWA = cfg.n_nodes, cfg.n_cores, cfg.npc, cfg.nwa
    NWC = cfg.nwc
    HALF = N // 2
    assert HALF <= 32768 and N - HALF <= 32768
    src = edge_index[0].astype(np.int64)
    dst = edge_index[1].astype(np.int64)

    # reference degree includes the self-loop; the self-loop term itself is
    # applied on-device as a transpose of the core's own g rows.
    deg = (np.bincount(dst, minlength=N) + 1).astype(np.float32)

    core_of = dst // NPC
    wloc = (dst - core_of * NPC) // WIN
    half_of = (src >= HALF).astype(np.int64)

    keys = (core_of * NWA + wloc) * 2 + half_of
    order = np.argsort(keys, kind="stable")
    s_sorted = src[order]
    d_sorted = dst[order]
    counts = np.bincount(keys[order], minlength=C * NWA * 2).reshape(C, NWA, 2)
    starts = np.zeros(C * NWA * 2 + 1, dtype=np.int64)
    np.cumsum(counts.reshape(-1), out=starts[1:])

    # tiles per (window, half), equalized across cores (single SPMD program)
    nt2 = np.ceil(counts / P).astype(np.int64).max(axis=0)        # [NWA, 2]
    meta = dict(nt=[], tile_base=[], T=[], chunks=[])
    per_core = dict(idx16=[], dst_rel=[])
    ch_cols = CHUNK_TOK // P
    for h in range(2):
        nt = nt2[:, h]
        tile_base = np.zeros(NWA + 1, dtype=np.int64)
        np.cumsum(nt, out=tile_base[1:])
        T = int(tile_base[-1])
        L = T * P
        idx16 = np.zeros((C, 128, max(1, T * 8)), dtype=np.int16)
        dst_rel = np.full((C, P, max(1, T)), -1.0, dtype=np.float32)
        for c in range(C):
            flat_idx = np.zeros(max(16, L), dtype=np.int16)
            for w in range(NWA):
                cnt = int(counts[c, w, h])
                if cnt == 0:
                    continue
                e0 = int(starts[(c * NWA + w) * 2 + h])
                tok = tile_base[w] * P + np.arange(cnt)
                flat_idx[tok] = (s_sorted[e0:e0 + cnt] - h * HALF).astype(
                    np.int16)
                dst_rel[c, tok % P, tok // P] = (
                    d_sorted[e0:e0 + cnt] - c * NPC - w * WIN
                ).astype(np.float32)
            if L > 0:
                wrapped = flat_idx[:L].reshape(L // 16, 16).T      # [16, L/16]
                idx16[c] = np.tile(wrapped, (8, 1))
        meta["nt"].append(nt.tolist())
        meta["tile_base"].append(tile_base.tolist())
        meta["T"].append(T)
        meta["chunks"].append(
            [(c0, min(c0 + ch_cols, T)) for c0 in range(0, T, ch_cols)])
        per_core["idx16"].append(idx16)
        per_core["dst_rel"].append(dst_rel)

    # batch ids per own node, node-major [P, NWC], -1 for tail padding
    batch_col = np.full((C, P, NWC), -1.0, dtype=np.float32)
    for c in range(C):
        own = batch[c * NPC:(c + 1) * NPC].astype(np.float32)
        n = np.arange(NPC)
        batch_col[c, n % P, n // P] = own

    # deg for own nodes in both layouts (pad with 1.0 -> dinv finite)
    deg_col = np.ones((C, P, NWC), dtype=np.float32)
    deg_row = np.ones((C, 1, NWA * WIN), dtype=np.float32)
    for c in range(C):
        own = deg[c * NPC:(c + 1) * NPC]
        n = np.arange(NPC)
        deg_col[c, n % P, n // P] = own
        deg_row[c, 0, :NPC] = own

    meta["half"] = HALF
    per_core.update(batch_col=batch_col, deg_col=deg_col, deg_row=deg_row)
    return meta, per_core


# ---------------------------------------------------------------------------
# Bass program
# ---------------------------------------------------------------------------

def build_program(cfg: Cfg, meta, debug_outputs: bool = False):
    N, C, NPC, G = cfg.n_nodes, cfg.n_cores, cfg.npc, cfg.n_graphs
    NWA, NWC = cfg.nwa, cfg.nwc
    HALF = meta["half"]
    T2 = meta["T"]
    nt2 = meta["nt"]
    tile_base2 = meta["tile_base"]
    chunks2 = meta["chunks"]
    f32 = mybir.dt.float32
    i16 = mybir.dt.int16
    FT = mybir.ActivationFunctionType
    ALU = mybir.AluOpType
    GI = max(G, WIN)                      # iota width for S builds + pooling
    ch_cols = CHUNK_TOK // P

    nc = bacc.Bacc("TRN2", target_bir_lowering=False, debug=False,
                   num_devices=C, num_swdge_queues=4)

    # ---- I/O ----
    xT_d = nc.dram_tensor("xT", [F, NPC], f32, kind="ExternalInput")
    W1_d = nc.dram_tensor("W1", [F, F], f32, kind="ExternalInput")
    W2_d = nc.dram_tensor("W2", [F, F], f32, kind="ExternalInput")
    b1_d = nc.dram_tensor("b1", [F, 1], f32, kind="ExternalInput")
    b2_d = nc.dram_tensor("b2", [F, 1], f32, kind="ExternalInput")
    woutf_d = nc.dram_tensor("wout_f", [F, 1], f32, kind="ExternalInput")
    wlast_d = nc.dram_tensor("wlast", [1, 1], f32, kind="ExternalInput")
    bout_d = nc.dram_tensor("bout", [1, 1], f32, kind="ExternalInput")
    depth_d = nc.dram_tensor("depth_row", [1, G], f32, kind="ExternalInput")
    degc_d = nc.dram_tensor("deg_col", [P, NWC], f32, kind="ExternalInput")
    degr_d = nc.dram_tensor("deg_row", [1, NWA * WIN], f32,
                            kind="ExternalInput")
    iota_d = nc.dram_tensor("iota_all", [P, GI], f32, kind="ExternalInput")
    iotar_d = nc.dram_tensor("iota_rep", [P, SBATCH * WIN], f32,
                             kind="ExternalInput")
    idx_d = [nc.dram_tensor(f"idx16_{h}", [128, max(1, T2[h] * 8)], i16,
                            kind="ExternalInput") for h in range(2)]
    drel_d = [nc.dram_tensor(f"dst_rel_{h}", [P, max(1, T2[h])], f32,
                             kind="ExternalInput") for h in range(2)]
    bcol_d = nc.dram_tensor("batch_col", [P, NWC], f32, kind="ExternalInput")
    y_d = nc.dram_tensor("y_out", [1, G], f32, kind="ExternalOutput")
    dbg = {}
    if debug_outputs:
        dbg["h1T"] = nc.dram_tensor("dbg_h1T", [F, NPC], f32,
                                    kind="ExternalOutput")
        dbg["h2T"] = nc.dram_tensor("dbg_h2T", [F, NPC], f32,
                                    kind="ExternalOutput")
        dbg["g1"] = nc.dram_tensor("dbg_g1", [N, F], f32,
                                   kind="ExternalOutput")
        dbg["pool"] = nc.dram_tensor("dbg_pool", [2, G], f32,
                                     kind="ExternalOutput")

    with tile.TileContext(nc) as tc:
        with (
            tc.tile_pool(name="const", bufs=1) as const_pool,
            tc.tile_pool(name="big", bufs=1) as big_pool,
            tc.tile_pool(name="gbuf", bufs=8) as gbuf_pool,
            tc.tile_pool(name="work", bufs=2) as work_pool,
            tc.tile_pool(name="spool", bufs=3) as s_pool,
            tc.tile_pool(name="psA", bufs=3, space="PSUM") as psumA,
            tc.tile_pool(name="psB", bufs=2, space="PSUM") as psumB,
            tc.tile_pool(name="psC", bufs=1, space="PSUM") as psumC,
            tc.tile_pool(name="dram", bufs=1, space="DRAM") as dram_pool,
        ):
            # ---- load constants ----
            def load(pool, dram_t, shape, dtype=f32, name=None):
                t = pool.tile(shape, dtype, name=name or dram_t.name + "_sb")
                nc.sync.dma_start(t[:], dram_t[:])
                return t

            xT = load(big_pool, xT_d, [F, NPC])
            W1 = load(const_pool, W1_d, [F, F])
            W2 = load(const_pool, W2_d, [F, F])
            b1 = load(const_pool, b1_d, [F, 1])
            b2 = load(const_pool, b2_d, [F, 1])
            woutf = load(const_pool, woutf_d, [F, 1])
            wlast = load(const_pool, wlast_d, [1, 1])
            bout = load(const_pool, bout_d, [1, 1])
            depth = load(const_pool, depth_d, [1, G])
            deg_col = load(const_pool, degc_d, [P, NWC])
            deg_row = load(const_pool, degr_d, [1, NWA * WIN])
            iota = load(const_pool, iota_d, [P, GI])
            iota_rep = load(const_pool, iotar_d, [P, SBATCH * WIN])
            idx_sb = [load(big_pool, idx_d[h], [128, max(1, T2[h] * 8)],
                           i16, name=f"idx_sb{h}") for h in range(2)]
            drel = [load(big_pool, drel_d[h], [P, max(1, T2[h])],
                         name=f"drel_sb{h}") for h in range(2)]
            bcol = load(const_pool, bcol_d, [P, NWC])

            ones1F = const_pool.tile([1, F], f32, name="ones1F")
            nc.vector.memset(ones1F[:], 1.0)
            ident = const_pool.tile([P, P], f32, name="ident")
            make_identity(nc, ident[:])

            # ---- dinv: node-major for the linear phase ----
            dinv_col = const_pool.tile([P, NWC], f32, name="dinv_col")
            nc.scalar.activation(dinv_col[:], deg_col[:], FT.Sqrt)
            nc.vector.reciprocal(dinv_col[:], dinv_col[:])

            dinvT = big_pool.tile([F, NWA * WIN], f32, name="dinvT")

            def build_dinvT():
                for j0 in range(0, NWA * WIN, 512):
                    j1 = min(j0 + 512, NWA * WIN)
                    ps = psumB.tile([F, 512], f32, name="bc_ps", tag="psB")
                    nc.tensor.matmul(ps[:, : j1 - j0], ones1F[:],
                                     deg_row[:, j0:j1], start=True, stop=True)
                    nc.scalar.activation(dinvT[:, j0:j1], ps[:, : j1 - j0],
                                         FT.Sqrt)
                    nc.vector.reciprocal(dinvT[:, j0:j1], dinvT[:, j0:j1])

            # ---- internal DRAM gather tables ----
            g1_own = dram_pool.tile([NPC, F], f32, name="g1_own")
            g2_own = dram_pool.tile([NPC, F], f32, name="g2_own")
            g1_full = dram_pool.tile([N, F], f32, name="g1_full",
                                     addr_space="Shared")
            g2_full = dram_pool.tile([N, F], f32, name="g2_full",
                                     addr_space="Shared")
            cc_in = dram_pool.tile([2, G], f32, name="cc_in")
            cc_out = dram_pool.tile([2, G], f32, name="cc_out",
                                    addr_space="Shared")

            # ---- linear phase: g_own = dinv * (h @ W), h given transposed
            def linear_phase(hT, W, g_own_dram):
                for w in range(NWC):
                    n0 = w * P
                    n1 = min(n0 + P, NPC)
                    m = n1 - n0
                    ps = psumA.tile([P, F], f32, name="lin_ps", tag="psA")
                    nc.tensor.matmul(ps[:m, :], hT[:, n0:n1], W[:],
                                     start=True, stop=True)
                    gt = work_pool.tile([P, F], f32, name="lin_g")
                    nc.vector.tensor_scalar(gt[:m, :], ps[:m, :],
                                            dinv_col[:m, w:w + 1], None,
                                            op0=ALU.mult)
                    nc.sync.dma_start(g_own_dram[n0:n1, :], gt[:m, :])

            def allgather(g_own, g_full):
                nc.gpsimd.collective_compute(
                    "AllGather", ALU.bypass,
                    replica_groups=[list(range(C))],
                    ins=[g_own.opt()], outs=[g_full.opt()],
                )

            # ---- pooling window op (layer 2 only), per 128-node subwindow
            pool_ps = psumC.tile([2, G], f32, name="pool_ps", tag="psC")

            pool_sel = [None]

            def pool_sub(h2T, s):
                n0 = s * P
                m = min(P, NPC - n0)
                sc_ps = psumB.tile([P, 1], f32, name="score_ps", tag="psB")
                nc.tensor.matmul(sc_ps[:m, :], h2T[:, n0:n0 + m], woutf[:],
                                 start=True, stop=True)
                sc = work_pool.tile([P, 2], f32, name="score_sb")
                nc.vector.memset(sc[:], 0.0)
                nc.vector.memset(sc[:m, 1:2], 1.0)
                nc.vector.tensor_copy(sc[:m, 0:1], sc_ps[:m, :])
                if s % 2 == 0:
                    nbp = min(2, NWC - s)
                    sg = work_pool.tile([P, 2 * G], f32, name="sel_pool")
                    nc.vector.tensor_tensor(
                        sg[:, :nbp * G].rearrange("p (t j) -> p t j", j=G),
                        iota[:, :G].rearrange(
                            "p (o j) -> p o j", o=1).to_broadcast(
                                (P, nbp, G)),
                        bcol[:, s:s + nbp].rearrange(
                            "p (t o) -> p t o", o=1).to_broadcast(
                                (P, nbp, G)),
                        op=ALU.is_equal)
                    pool_sel[0] = sg
                sg = pool_sel[0]
                nc.tensor.matmul(pool_ps[:], sc[:],
                                 sg[:, (s % 2) * G:(s % 2) * G + G],
                                 start=(s == 0), stop=(s == NWC - 1))

            # ---- aggregation: hT = relu(dinv * (scatter_add(g[src]) + g_own)
            #      + b), where the g_own term is the self-loop contribution.
            qrr = [0]

            def agg_phase(g_full, g_own, b_tile, hT_out, suffix, do_pool):
                tabs = [g_full[0:HALF, :], g_full[HALF:N, :]]
                chunk_tiles = {}

                def ensure_chunk(h, ci):
                    key = (h, ci)
                    if key in chunk_tiles:
                        return chunk_tiles[key]
                    c0, c1 = chunks2[h][ci]
                    ntok = (c1 - c0) * P
                    ct = gbuf_pool.tile([P, ch_cols * F], f32,
                                        name="chunk_" + suffix, tag="chunk")
                    nc.gpsimd.dma_gather(
                        out_ap=ct[:, :(c1 - c0) * F].rearrange(
                            "p (s e) -> p s e", e=F),
                        in_ap=tabs[h],
                        idxs_ap=idx_sb[h][:, c0 * 8:c1 * 8],
                        num_idxs=ntok,
                        num_idxs_reg=ntok,
                        elem_size=F,
                        single_packet=False,
                        queue_num=qrr[0] % 4,
                    )
                    qrr[0] += 1
                    chunk_tiles[key] = (ct, c0)
                    return chunk_tiles[key]

                for w in range(NWA):
                    n0 = w * WIN
                    m = min(WIN, NPC - n0)
                    n_tr = (m + P - 1) // P
                    ps = psumA.tile([F, WIN], f32, name="agg_ps_" + suffix,
                                    tag="psA")
                    nmm = nt2[0][w] + nt2[1][w] + n_tr
                    i = 0
                    for h in range(2):
                        ntw = nt2[h][w]
                        base = tile_base2[h][w]
                        for b0 in range(0, ntw, SBATCH):
                            nb = min(SBATCH, ntw - b0)
                            sw = s_pool.tile([P, nb * WIN], f32,
                                             name="sel_" + suffix, tag="sel",
                                             padded_shape=[P, SBATCH * WIN])
                            nc.vector.tensor_tensor(
                                sw[:].rearrange("p (t j) -> p t j", j=WIN),
                                iota_rep[:, :nb * WIN].rearrange(
                                    "p (t j) -> p t j", j=WIN),
                                drel[h][:, base + b0:base + b0 + nb].rearrange(
                                    "p (t o) -> p t o", o=1).to_broadcast(
                                        (P, nb, WIN)),
                                op=ALU.is_equal)
                            for t in range(nb):
                                gt = base + b0 + t
                                ct, c0 = ensure_chunk(h, gt // ch_cols)
                                col = gt - c0
                                nc.tensor.matmul(
                                    ps[:], ct[:, col * F:(col + 1) * F],
                                    sw[:, t * WIN:(t + 1) * WIN],
                                    start=(i == 0), stop=False)
                                i += 1
                    # self-loop term: transpose own g rows into the window
                    for s in range(n_tr):
                        ms = min(P, m - s * P)
                        gsl = work_pool.tile([P, F], f32,
                                             name="gself_" + suffix)
                        nc.sync.dma_start(
                            gsl[:ms, :], g_own[n0 + s * P:n0 + s * P + ms, :])
                        i += 1
                        nc.tensor.matmul(
                            ps[:, s * P:s * P + ms], gsl[:ms, :],
                            ident[:ms, :ms], is_transpose=True,
                            start=False, stop=(i == nmm))
                    tmp = work_pool.tile([F, WIN], f32, name="fin_" + suffix)
                    nc.vector.tensor_tensor(
                        tmp[:, :m], ps[:, :m], dinvT[:, n0:n0 + m],
                        op=ALU.mult)
                    nc.scalar.activation(hT_out[:, n0:n0 + m], tmp[:, :m],
                                         FT.Relu, bias=b_tile[:])
                    if do_pool:
                        for s in range(n0 // P, (n0 + m + P - 1) // P):
                            pool_sub(hT_out, s)

            h1T = big_pool.tile([F, NPC], f32, name="h1T")
            h2T = big_pool.tile([F, NPC], f32, name="h2T", tag="xT_sb")

            # ---- layer 1 ----
            linear_phase(xT, W1, g1_own)
            allgather(g1_own, g1_full)
            build_dinvT()
            agg_phase(g1_full, g1_own, b1, h1T, "l1", do_pool=False)

            # ---- layer 2 (pooling fused into the window loop) ----
            linear_phase(h1T, W2, g2_own)
            allgather(g2_own, g2_full)
            agg_phase(g2_full, g2_own, b2, h2T, "l2", do_pool=True)

            # ---- pooled sums/counts AllReduce + head ----
            pool_sb = const_pool.tile([2, G], f32, name="pool_sb")
            nc.vector.tensor_copy(pool_sb[:], pool_ps[:])
            nc.sync.dma_start(cc_in[:], pool_sb[:])
            nc.gpsimd.collective_compute(
                "AllReduce", ALU.add, replica_groups=[list(range(C))],
                ins=[cc_in.opt()], outs=[cc_out.opt()])
            pool_g0 = const_pool.tile([1, G], f32, name="pool_g0")
            pool_g1 = const_pool.tile([1, G], f32, name="pool_g1")
            nc.sync.dma_start(pool_g0[:], cc_out[0:1, :])
            nc.sync.dma_start(pool_g1[:], cc_out[1:2, :])
            if debug_outputs:
                nc.sync.dma_start(dbg["pool"][:], cc_out[:])

            # y = sums/max(cnt,1) + depth*wlast + bout
            cnt = const_pool.tile([1, G], f32, name="cnt_row")
            nc.vector.tensor_scalar(cnt[:], pool_g1[:], 1.0, None,
                                    op0=ALU.max)
            nc.vector.reciprocal(cnt[:], cnt[:])
            y = const_pool.tile([1, G], f32, name="y_row")
            nc.vector.tensor_tensor(y[:], pool_g0[:], cnt[:], op=ALU.mult)
            dterm = const_pool.tile([1, G], f32, name="dterm")
            nc.vector.tensor_scalar(dterm[:], depth[:], wlast[:], None,
                                    op0=ALU.mult)
            nc.vector.tensor_tensor(y[:], y[:], dterm[:], op=ALU.add)
            nc.vector.tensor_scalar(y[:], y[:], bout[:], None, op0=ALU.add)
            nc.sync.dma_start(y_d[:], y[:])

            if debug_outputs:
                nc.sync.dma_start(dbg["h1T"][:], h1T[:])
                nc.sync.dma_start(dbg["h2T"][:], h2T[:])
                nc.gpsimd.dma_start(dbg["g1"][:], g1_full[:])

    nc.compile()
    return nc


# ---------------------------------------------------------------------------
# full pipeline
# ---------------------------------------------------------------------------

def make_in_maps(cfg: Cfg, meta, per_core, x, depth, W1, b1, W2, b2, Wout,
                 bout):
    C, NPC, G = cfg.n_cores, cfg.npc, cfg.n_graphs
    GI = max(G, WIN)
    iota = np.broadcast_to(np.arange(GI, dtype=np.float32), (P, GI)).copy()
    in_maps = []
    for c in range(C):
        xT = np.ascontiguousarray(x[c * NPC:(c + 1) * NPC, :].T)
        in_maps.append({
            "xT": xT,
            "W1": np.ascontiguousarray(W1),
            "W2": np.ascontiguousarray(W2),
            "b1": b1.reshape(F, 1).copy(),
            "b2": b2.reshape(F, 1).copy(),
            "wout_f": Wout[:F, :].copy(),
            "wlast": Wout[F:, :].copy(),
            "bout": bout.reshape(1, 1).copy(),
            "depth_row": depth.reshape(1, G).copy(),
            "deg_col": per_core["deg_col"][c],
            "deg_row": per_core["deg_row"][c],
            "iota_all": iota,
            "iota_rep": np.tile(np.arange(WIN, dtype=np.float32),
                                (P, SBATCH)).reshape(P, SBATCH * WIN),
            "idx16_0": per_core["idx16"][0][c],
            "idx16_1": per_core["idx16"][1][c],
            "dst_rel_0": per_core["dst_rel"][0][c],
            "dst_rel_1": per_core["dst_rel"][1][c],
            "batch_col": per_core["batch_col"][c],
        })
    return in_maps


def kernel(x, edge_index, batch, depth, W1, b1, W2, b2, Wout, bout):
    cfg = Cfg()
    x = np.asarray(x, dtype=np.float32)
    edge_index = np.asarray(edge_index)
    batch = np.asarray(batch)
    depth = np.asarray(depth, dtype=np.float32)
    W1 = np.asarray(W1, dtype=np.float32)
    b1 = np.asarray(b1, dtype=np.float32)
    W2 = np.asarray(W2, dtype=np.float32)
    b2 = np.asarray(b2, dtype=np.float32)
    Wout = np.asarray(Wout, dtype=np.float32)
    bout = np.asarray(bout, dtype=np.float32)

    meta, per_core = host_prep(cfg, edge_index, batch)
    nc = build_program(cfg, meta)
    in_maps = make_in_maps(cfg, meta, per_core, x, depth, W1, b1, W2, b2,
                           Wout, bout)
    from concourse import bass_utils
    res = bass_utils.run_bass_kernel_spmd(
        nc, in_maps, core_ids=list(range(cfg.n_cores)))
    y = np.asarray(res.results[0]["y_out"]).reshape(cfg.n_graphs)
    return y.astype(np.float32)


if __name__ == "__main__":
    sys.path.insert(0, os.path.dirname(os.path.abspath(__file__)))
    import reference
    inputs = {k: np.asarray(v) for k, v in reference.setup_inputs().items()}
    out = kernel(**inputs)
    print("kernel output:", out[:8])
